# revision 2
# baseline (speedup 1.0000x reference)
"""Trainium2 Bass kernel for ConditionedSparseAttention (fp8-DoubleRow rev).

Problem: B=2, T_IN=2048, T_COND=1024 (S=3072), D=1024, H=16, HD=64, W=512.
The window mask depends only on end_inds[b]: every query attends to the same
1024 keys, so attention is a softmax over a fixed 1024-key set.

Sharding: 8 cores = 2 batches x 4 head-groups of 4 heads (as baseline).

This revision exploits the cost model / HW property that fp8e4m3 matmuls in
DoubleRow perf mode process 2 contraction rows per output-column cycle:
  - Q/K/V projections run as 3-term fp8 DR groups over host-prepared hi/lo
    fp8 planes of both X and W:  W@X ~= Whi@X8 + Wlo@X8 + Whi@Xres
    (error ~eps^2, cost 3/4 of bf16).
  - scores run as a single DR matmul per (head, kc, qtile) with contraction
    128 = 64 dims x {q8, qres}: s = k8.(q8+qres) = k8.q exactly; the only
    fp8 error left is k8's quantization (~1% on the final output).
    Layout trick: heads pair up in 128-partition tiles (A: heads 0,1;
    B: heads 2,3), slot dim 2 holds {q8, qres} / {k8, k8 dup}.
  - A@V and the output projection stay bf16 (fp8 there costs ~2.4% each).
Softmax exp splits across engines: ACT does exp directly from PSUM for some
heads; for the rest DVE stages scaled scores to SBUF (f16) and Pool (GPSIMD)
computes pow(e, s) -- exact in fp32 -- freeing ACT to absorb most PSUM->SBUF
drains (y, av, ot, V) as activation-Copy ops.

Scales (exactness preserved, folded out on host / in exp):
  wq *= 0.125*32, wk *= 32  -> scores_psum = 1024*s_true, exp scale 2^-10.
  wv *= 32, v_aug ones column = 32.0 -> reciprocal folds the 1/32 back.
Biases handled exactly as baseline (k-bias dropped, q-bias via cexp on
v_aug, v/out-bias folded on host).
"""
import os
import sys
import tempfile

os.environ["NEURON_COMPILE_CACHE_URL"] = tempfile.mkdtemp(prefix="bass_kernel_cache_")

try:
    import concourse  # noqa: F401
except ImportError:
    sys.path.insert(0, "/opt/trn_rl_repo")

import numpy as np
import ml_dtypes

import concourse.bacc as bacc
import concourse.tile as tile
import concourse.mybir as mybir
from concourse.bass_utils import run_bass_kernel_spmd

# ---- problem constants (hardcoded per harness contract) ----
B, T_IN, T_COND, D, H, HD, W = 2, 2048, 1024, 1024, 16, 64, 512
S = T_IN + T_COND            # 3072
SEL = 2 * W                  # 1024 selected keys
NH = 4                       # heads per core
NG = H // NH                 # 4 head groups
NCH = D // 128               # 8 input d-chunks
KT = SEL // 128              # 8 key tiles
QT = S // 128                # 24 query tiles
NSLAB = S // 512             # 6 query slabs
BF16 = mybir.dt.bfloat16
F16 = mybir.dt.float16
F32 = mybir.dt.float32
FP8 = mybir.dt.float8e4
AF = mybir.ActivationFunctionType
ALU = mybir.AluOpType
DR = mybir.MatmulPerfMode.DoubleRow
E4 = ml_dtypes.float8_e4m3

CQ = 32.0                   # extra scale on wq (beyond 0.125)
CK = 32.0                   # scale on wk
CV = 32.0                   # scale on wv; ones column = CV so rec folds it
S_INV = 1.0 / (CQ * CK)     # exp input scale
POOL_HEADS = (0, 1, 2)      # heads whose exp runs DVE-stage + Pool pow
N_WARM = 40                 # PE warmup matmuls

_CACHE = {}


def _build():
    if "nc" in _CACHE:
        return _CACHE["nc"]

    nc = bacc.Bacc("TRN2", target_bir_lowering=False, debug=False,
                   enable_asserts=True, num_devices=8)

    xt8_d = nc.dram_tensor("xt8", (128, NCH, S), FP8, kind="ExternalInput").ap()
    xtr_d = nc.dram_tensor("xtr", (128, NCH, S), FP8, kind="ExternalInput").ap()
    xs8_d = nc.dram_tensor("xs8", (128, NCH, SEL), FP8, kind="ExternalInput").ap()
    xsr_d = nc.dram_tensor("xsr", (128, NCH, SEL), FP8, kind="ExternalInput").ap()
    whiq_d = nc.dram_tensor("whiq", (128, NCH, 256), FP8, kind="ExternalInput").ap()
    wloq_d = nc.dram_tensor("wloq", (128, NCH, 256), FP8, kind="ExternalInput").ap()
    whik_d = nc.dram_tensor("whik", (128, NCH, 256), FP8, kind="ExternalInput").ap()
    wlok_d = nc.dram_tensor("wlok", (128, NCH, 256), FP8, kind="ExternalInput").ap()
    whiv_d = nc.dram_tensor("whiv", (128, NCH, 256), FP8, kind="ExternalInput").ap()
    wlov_d = nc.dram_tensor("wlov", (128, NCH, 256), FP8, kind="ExternalInput").ap()
    wo_d = nc.dram_tensor("wo", (128, 2, D), BF16, kind="ExternalInput").ap()
    cexp_d = nc.dram_tensor("cexp", (128, KT, NH), F32, kind="ExternalInput").ap()
    ident_d = nc.dram_tensor("ident", (128, 128), BF16, kind="ExternalInput").ap()
    y_d = nc.dram_tensor("y", (128, NCH, S), BF16, kind="ExternalOutput").ap()

    with tile.TileContext(nc) as tc:
        with (
            tc.tile_pool(name="const", bufs=1) as cpool,
            tc.tile_pool(name="work", bufs=1) as work,
            tc.tile_pool(name="exps", bufs=6) as epool,
            tc.tile_pool(name="sxp", bufs=4) as sxpool,
            tc.tile_pool(name="osb", bufs=2) as opool,
            tc.tile_pool(name="ysb", bufs=2) as ypool,
            tc.tile_pool(name="ps_s", bufs=2, space="PSUM") as ps_s,   # scores 2x2 banks
            tc.tile_pool(name="ps_qp", bufs=1, space="PSUM") as ps_qp,  # 1 bank
            tc.tile_pool(name="ps_op", bufs=2, space="PSUM") as ps_op,  # 2 banks
            tc.tile_pool(name="ps_av", bufs=1, space="PSUM") as ps_av,  # 1 bank
        ):
            # ---------- input DMAs (SP queue) -- K path first ---------------
            whik = cpool.tile([128, NCH, 256], FP8, tag="whik")
            wlok = cpool.tile([128, NCH, 256], FP8, tag="wlok")
            xs8 = cpool.tile([128, NCH, SEL], FP8, tag="xs8")
            xsr = cpool.tile([128, NCH, SEL], FP8, tag="xsr")
            whiq = cpool.tile([128, NCH, 256], FP8, tag="whiq")
            wloq = cpool.tile([128, NCH, 256], FP8, tag="wloq")
            whiv = cpool.tile([128, NCH, 256], FP8, tag="whiv")
            wlov = cpool.tile([128, NCH, 256], FP8, tag="wlov")
            xt8 = cpool.tile([128, NCH, S], FP8, tag="xt8")
            xtr = cpool.tile([128, NCH, S], FP8, tag="xtr")
            wo = cpool.tile([128, 2, D], BF16, tag="wo")
            cexp = cpool.tile([128, KT, NH], F32, tag="cexp")
            ident = cpool.tile([128, 128], BF16, tag="ident")

            nc.sync.dma_start(whik[:], whik_d[:])
            nc.sync.dma_start(xs8[:, :, 0:512], xs8_d[:, :, 0:512])
            nc.sync.dma_start(wlok[:], wlok_d[:])
            nc.sync.dma_start(xs8[:, :, 512:1024], xs8_d[:, :, 512:1024])
            nc.sync.dma_start(xsr[:, :, 0:512], xsr_d[:, :, 0:512])
            nc.sync.dma_start(xsr[:, :, 512:1024], xsr_d[:, :, 512:1024])
            nc.sync.dma_start(whiq[:], whiq_d[:])
            nc.sync.dma_start(xt8[:, :, 0:512], xt8_d[:, :, 0:512])
            nc.sync.dma_start(wloq[:], wloq_d[:])
            nc.sync.dma_start(xtr[:, :, 0:512], xtr_d[:, :, 0:512])
            nc.sync.dma_start(whiv[:], whiv_d[:])
            nc.sync.dma_start(wlov[:], wlov_d[:])
            nc.sync.dma_start(cexp[:], cexp_d[:])
            nc.sync.dma_start(wo[:], wo_d[:])
            nc.sync.dma_start(ident[:], ident_d[:])
            for sl in range(1, NSLAB):
                nc.sync.dma_start(xt8[:, :, 512 * sl:512 * (sl + 1)],
                                  xt8_d[:, :, 512 * sl:512 * (sl + 1)])
                nc.sync.dma_start(xtr[:, :, 512 * sl:512 * (sl + 1)],
                                  xtr_d[:, :, 512 * sl:512 * (sl + 1)])

            # ---------- persistent tensors ----------
            ktab = [work.tile([128, 2, SEL], FP8, tag=f"kt{i}", name=f"kt{i}")
                    for i in range(2)]                     # slot0=k8, slot1=dup
            qtab = [work.tile([128, 2, S], FP8, tag=f"qt{i}", name=f"qt{i}")
                    for i in range(2)]                     # slot0=q8, slot1=res
            ot = work.tile([128, 2, S], BF16, tag="ot")    # O^T
            v_aug = [work.tile([128, NH, HD + 1], BF16, tag=f"va{kc}",
                               name=f"va{kc}") for kc in range(KT)]
            ebase = work.tile([128, NH, 128], F32, tag="eb")   # e for pow
            zt = work.tile([128, 128], BF16, tag="zt")         # warmup src

            nc.gpsimd.memset(zt[:], 0.0)
            nc.gpsimd.memset(ebase[:], float(np.e))
            for kc in range(KT):
                nc.gpsimd.memset(v_aug[kc][:], CV)

            # ---------- PE warmup: keep PE busy from ~0.5us ----------------
            wm = ps_av.tile([128, NH, HD + 1], F32, tag="av", name="wm")
            for i in range(N_WARM):
                nc.tensor.matmul(wm[:, i % NH, 0:HD], zt[:], zt[:, 0:HD],
                                 start=True, stop=True)

            # ---------- 3-term fp8-DR projection groups --------------------
            def k_proj_group(t, half, pool, tag):
                psk = pool.tile([128, 512], F32, tag=tag, name=f"kp{t}_{half}")
                cols = slice(512 * half, 512 * (half + 1))
                mms = [(w, x) for (w, x) in ((whik, xs8), (wlok, xs8), (whik, xsr))
                       for _ in range(1)]
                idx = 0
                for (wt, xx) in ((whik, xs8), (wlok, xs8), (whik, xsr)):
                    for c in range(NCH // 2):
                        nc.tensor.matmul(
                            psk[:], wt[:, 2 * c:2 * c + 2, 128 * t:128 * (t + 1)],
                            xx[:, 2 * c:2 * c + 2, cols],
                            start=(idx == 0), stop=(idx == 11), perf_mode=DR)
                        idx += 1
                # drains: k8 (DVE, psum->fp8) + dup (DVE, sbuf->sbuf)
                nc.vector.tensor_copy(ktab[t][:, 0, cols], psk[:])
                nc.vector.tensor_copy(ktab[t][:, 1, cols], ktab[t][:, 0, cols])

            def v_proj_group(kc):
                psv = ps_op.tile([128, 512], F32, tag="op", name=f"vp{kc}")
                idx = 0
                for (xx, wt) in ((xs8, whiv), (xsr, whiv), (xs8, wlov)):
                    for c in range(NCH // 2):
                        nc.tensor.matmul(
                            psv[:, 0:256], xx[:, 2 * c:2 * c + 2, 128 * kc:128 * (kc + 1)],
                            wt[:, 2 * c:2 * c + 2, :],
                            start=(idx == 0), stop=(idx == 11), perf_mode=DR)
                        idx += 1
                nc.scalar.copy(
                    v_aug[kc][:, :, 0:HD],
                    psv[:, 0:256].rearrange("p (h hd) -> p h hd", h=NH))
                for h in range(NH):
                    nc.gpsimd.tensor_scalar(
                        v_aug[kc][:, h, :], v_aug[kc][:, h, :],
                        cexp[:, kc, h:h + 1], None, ALU.mult)

            # Q-proj: 12 DR mms per (t, sl), emitted in chunks.
            qp_state = {}
            QP_TERMS = [(0, 0), (0, 1), (0, 2), (0, 3),    # (term, c): whiq x8
                        (1, 0), (1, 1), (1, 2), (1, 3),    # wloq x8
                        (2, 0), (2, 1), (2, 2), (2, 3)]    # whiq xres

            def q_proj_group(t, sl, lo, hi, pool=None, tag="qp"):
                key = (t, sl)
                if key not in qp_state:
                    qp_state[key] = (pool or ps_qp).tile(
                        [128, 512], F32, tag=tag, name=f"qp{t}_{sl}")
                psq = qp_state[key]
                cols = slice(512 * sl, 512 * (sl + 1))
                for idx in range(lo, hi):
                    term, c = QP_TERMS[idx]
                    wt = (whiq, wloq, whiq)[term]
                    xx = (xt8, xt8, xtr)[term]
                    nc.tensor.matmul(
                        psq[:], wt[:, 2 * c:2 * c + 2, 128 * t:128 * (t + 1)],
                        xx[:, 2 * c:2 * c + 2, cols],
                        start=(idx == 0), stop=(idx == 11), perf_mode=DR)
                if hi == 12:
                    nc.vector.tensor_copy(qtab[t][:, 0, cols], psq[:])
                    nc.vector.scalar_tensor_tensor(
                        qtab[t][:, 1, cols], psq[:], 1.0, qtab[t][:, 0, cols],
                        ALU.mult, ALU.subtract)
                    del qp_state[key]

            # Prologue: K fully (gates first scores), Q slab 0.
            k_proj_group(0, 0, ps_qp, "qp")
            k_proj_group(0, 1, ps_op, "op")
            k_proj_group(1, 0, ps_qp, "qp")
            k_proj_group(1, 1, ps_op, "op")
            q_proj_group(0, 0, 0, 12)
            q_proj_group(1, 0, 0, 12, pool=ps_op, tag="op")

            # ---------- main loop over 128-query tiles ----------------------
            o_prev = None

            def emit_transpose():
                o_sb_p, qtp = o_prev
                for w in range(2):
                    tp = ps_s.tile([128, 128], BF16, tag="S", name=f"tp{qtp}_{w}")
                    nc.tensor.transpose(
                        tp[:], o_sb_p[:, 2 * w:2 * w + 2, :]
                        .rearrange("p a b -> p (a b)"), ident[:])
                    nc.scalar.copy(ot[:, w, 128 * qtp:128 * (qtp + 1)], tp[:])

            def emit_oproj(dt, sl):
                pso = ps_op.tile([128, 512], F32, tag="op", name=f"op{dt}_{sl}")
                for t in range(2):
                    nc.tensor.matmul(
                        pso[:], wo[:, t, 128 * dt:128 * (dt + 1)],
                        ot[:, t, 512 * sl:512 * (sl + 1)],
                        start=(t == 0), stop=(t == 1))
                nc.scalar.copy(y_sb[:, dt, :], pso[:])
                if dt % 2 == 1:
                    nc.sync.dma_start(
                        y_d[:, dt - 1:dt + 1, 512 * sl:512 * (sl + 1)],
                        y_sb[:, dt - 1:dt + 1, :])

            def emit_exp(h, st, ex_t):
                if h in POOL_HEADS:
                    sx = sxpool.tile([128, KT, 128], F16, tag="sx",
                                     name=f"sx{h}")
                    for hf in range(2):
                        blk = slice(4 * hf, 4 * hf + 4)
                        nc.vector.tensor_scalar(
                            sx[:, blk, :], st[:, blk, :], S_INV, None, ALU.mult)
                        nc.gpsimd.tensor_tensor(
                            ex_t[:, blk, :], ebase[:], sx[:, blk, :], ALU.pow)
                else:
                    nc.scalar.activation(ex_t[:], st[:], AF.Exp, scale=S_INV)

            y_sb = None
            for qt in range(QT):
                sl, r = divmod(qt, 4)
                if r == 2 and qt >= 6:
                    y_sb = ypool.tile([128, NCH, 512], BF16, tag="ysb",
                                      name=f"ysb{sl}")

                # Q-proj of slab sl+1 in 6-mm chunks (2 chunks per group)
                qp_chunks = []
                if 4 <= qt < 4 * (NSLAB - 1):
                    t = r // 2
                    lo, hi = (0, 6) if r % 2 == 0 else (6, 12)
                    qp_chunks = [(t, sl + 1, lo, hi)]
                u = (qt - 6) // 4
                if u >= 0:
                    dts = ((4, 5), (6, 7), (0, 1), (2, 3))[r]
                    op_groups = [(dt, u) for dt in dts]
                else:
                    op_groups = []

                ex = [None] * NH
                av = ps_av.tile([128, NH, HD + 1], F32, tag="av", name=f"av{qt}")
                for h in range(NH):
                    X = h // 2
                    o = 64 * (h % 2)
                    st = ps_s.tile([128, KT, 128], F32, tag="S", name=f"s{qt}_{h}")
                    for kc in range(KT):
                        nc.tensor.matmul(
                            st[:, kc, :],
                            ktab[X][o:o + 64, :, 128 * kc:128 * (kc + 1)],
                            qtab[X][o:o + 64, :, 128 * qt:128 * (qt + 1)],
                            start=True, stop=True, perf_mode=DR)
                    ex[h] = epool.tile([128, KT, 128], BF16, tag="ex",
                                       name=f"ex{qt}_{h}")
                    emit_exp(h, st, ex[h])

                    if qt == 0:
                        for kc in {1: (0,), 2: (1, 2), 3: (3,)}.get(h, ()):
                            v_proj_group(kc)
                        continue
                    if qt == 1 and h == 1:
                        q_proj_group(0, 1, 0, 12)
                    if h == 2 and op_groups:
                        emit_oproj(*op_groups[0])
                    if qt == 1 and h == 3:
                        q_proj_group(1, 1, 0, 12)
                    if h == 3 and qp_chunks:
                        q_proj_group(*qp_chunks[0])
                    if h >= 1:
                        hh = h - 1
                        for kc in range(KT):
                            nc.tensor.matmul(
                                av[:, hh, :], ex[hh][:, kc, :], v_aug[kc][:, hh, :],
                                start=(kc == 0), stop=(kc == KT - 1))
                if qt == 0:
                    for kc in range(4, KT):
                        v_proj_group(kc)
                    for hh in range(NH - 1):
                        for kc in range(KT):
                            nc.tensor.matmul(
                                av[:, hh, :], ex[hh][:, kc, :], v_aug[kc][:, hh, :],
                                start=(kc == 0), stop=(kc == KT - 1))
                for kc in range(KT):
                    nc.tensor.matmul(
                        av[:, NH - 1, :], ex[NH - 1][:, kc, :],
                        v_aug[kc][:, NH - 1, :],
                        start=(kc == 0), stop=(kc == KT - 1))
                for g in op_groups[1:]:
                    emit_oproj(*g)
                if o_prev is not None:
                    emit_transpose()

                # normalize: av -> SBUF (ACT), recip (DVE), scale (Pool).
                av_sb = opool.tile([128, NH, HD + 1], F32, tag="avsb",
                                   name=f"avsb{qt}")
                nc.scalar.copy(av_sb[:], av[:])
                rec = opool.tile([128, NH], F32, tag="rec", name=f"rec{qt}")
                nc.vector.reciprocal(rec[:], av_sb[:, :, HD])
                o_sb = opool.tile([128, NH, HD], BF16, tag="osb", name=f"o{qt}")
                for h in range(NH):
                    nc.gpsimd.tensor_scalar(
                        o_sb[:, h, :], av_sb[:, h, 0:HD], rec[:, h:h + 1],
                        None, ALU.mult)
                o_prev = (o_sb, qt)

            # tail
            emit_transpose()
            for dt in (4, 5, 6, 7):
                emit_oproj(dt, NSLAB - 2)
            y_sb = ypool.tile([128, NCH, 512], BF16, tag="ysb", name="ysb_tail")
            for dt in range(NCH):
                emit_oproj(dt, NSLAB - 1)

    nc.compile()
    _CACHE["nc"] = nc
    return nc


def _to_pko(a2d, dt=ml_dtypes.bfloat16):
    """(D_in, M) row-major -> [128, D_in//128, M] with d = ko*128 + p."""
    d_in, m = a2d.shape
    return np.ascontiguousarray(
        a2d.reshape(d_in // 128, 128, m).transpose(1, 0, 2).astype(dt))


def _hi_lo(a2d):
    """fp8 hi/lo planes of (D_in, M) array in pko layout."""
    hi = _to_pko(a2d, E4)
    d_in, m = a2d.shape
    hi_f = hi.astype(np.float32)
    lo_f = _to_pko(a2d, np.float32) - hi_f
    return hi, np.ascontiguousarray(lo_f.astype(E4))


def kernel(x, condition, end_inds, in_proj_w, in_proj_b, out_w, out_b):
    nc = _build()

    x = np.asarray(x, dtype=np.float32)
    condition = np.asarray(condition, dtype=np.float32)
    end_inds = np.asarray(end_inds, dtype=np.int32)
    in_proj_w = np.asarray(in_proj_w, dtype=np.float32)
    in_proj_b = np.asarray(in_proj_b, dtype=np.float32)
    out_w = np.asarray(out_w, dtype=np.float32)
    out_b = np.asarray(out_b, dtype=np.float32)

    ident = np.eye(128, dtype=ml_dtypes.bfloat16)
    bo_eff = out_b + out_w @ in_proj_b[2 * D:3 * D]          # v-bias fold

    wq_g, wk_g, wv_g, wo_g, m_g = [], [], [], [], []
    for g in range(NG):
        rows = slice(256 * g, 256 * (g + 1))
        wq_raw = in_proj_w[rows]                              # (256, 1024)
        wk_raw = in_proj_w[D + 256 * g:D + 256 * (g + 1)]
        wv_raw = in_proj_w[2 * D + 256 * g:2 * D + 256 * (g + 1)]
        wq_g.append(_hi_lo(np.ascontiguousarray((0.125 * CQ * wq_raw).T)))
        wk_g.append(_hi_lo(np.ascontiguousarray((CK * wk_raw).T)))
        wv_g.append(_hi_lo(np.ascontiguousarray((CV * wv_raw).T)))
        wo_g.append(_to_pko(np.ascontiguousarray(out_w[:, rows].T)))
        m = np.zeros((D, NH), dtype=np.float32)
        for hl in range(NH):
            bq_h = 0.125 * in_proj_b[256 * g + 64 * hl:256 * g + 64 * hl + 64]
            m[:, hl] = wk_raw[64 * hl:64 * hl + 64].T @ bq_h
        m_g.append(m)

    in_maps = []
    xt_b, xst_b = [], []
    for b in range(B):
        inp = np.concatenate([x[b], condition[b]], axis=0)    # (3072, 1024)
        e = int(end_inds[b])
        sel = np.concatenate([inp[e - W:e], inp[T_IN + e - W:T_IN + e]], axis=0)
        xt_b.append(_hi_lo(np.ascontiguousarray(inp.T)))
        xst_b.append((sel, _hi_lo(np.ascontiguousarray(sel.T))))

    for core in range(8):
        b, g = divmod(core, NG)
        sel, (xs8, xsr) = xst_b[b]
        c = sel @ m_g[g]                                      # (1024, NH)
        cexp = np.exp(c).reshape(KT, 128, NH).transpose(1, 0, 2)
        whiq, wloq = wq_g[g]
        whik, wlok = wk_g[g]
        whiv, wlov = wv_g[g]
        xt8, xtr = xt_b[b]
        in_maps.append({
            "xt8": xt8, "xtr": xtr, "xs8": xs8, "xsr": xsr,
            "whiq": whiq, "wloq": wloq, "whik": whik, "wlok": wlok,
            "whiv": whiv, "wlov": wlov, "wo": wo_g[g],
            "cexp": np.ascontiguousarray(cexp.astype(np.float32)),
            "ident": ident,
        })

    res = run_bass_kernel_spmd(nc, in_maps, core_ids=list(range(8)))

    out = np.zeros((B, S, D), dtype=np.float32)
    for core in range(8):
        b, g = divmod(core, NG)
        yv = np.asarray(res.results[core]["y"]).astype(np.float32)  # [128,8,3072]
        out[b] += yv.transpose(1, 0, 2).reshape(D, S).T       # (3072, 1024)
    out += bo_eff[None, None, :]
    return out


# revision 23
# speedup vs baseline: 1.5043x; 1.5043x over previous
"""Trainium2 Bass kernel for ConditionedSparseAttention (fp8-DoubleRow rev).

Problem: B=2, T_IN=2048, T_COND=1024 (S=3072), D=1024, H=16, HD=64, W=512.
The window mask depends only on end_inds[b]: every query attends to the same
1024 keys, so attention is a softmax over a fixed 1024-key set.

Sharding: 8 cores = 2 batches x 4 head-groups of 4 heads (as baseline).

This revision exploits the cost model / HW property that fp8e4m3 matmuls in
DoubleRow perf mode process 2 contraction rows per output-column cycle:
  - Q/K/V projections run as 3-term fp8 DR groups over host-prepared hi/lo
    fp8 planes of both X and W:  W@X ~= Whi@X8 + Wlo@X8 + Whi@Xres
    (error ~eps^2, cost 3/4 of bf16).
  - scores run as a single DR matmul per (head, kc, qtile) with contraction
    128 = 64 dims x {q8, qres}: s = k8.(q8+qres) = k8.q exactly; the only
    fp8 error left is k8's quantization (~1% on the final output).
    Layout trick: heads pair up in 128-partition tiles (A: heads 0,1;
    B: heads 2,3), slot dim 2 holds {q8, qres} / {k8, k8 dup}.
  - A@V and the output projection stay bf16 (fp8 there costs ~2.4% each).
Softmax exp splits across engines: ACT does exp directly from PSUM for some
heads; for the rest DVE stages scaled scores to SBUF (f16) and Pool (GPSIMD)
computes pow(e, s) -- exact in fp32 -- freeing ACT to absorb most PSUM->SBUF
drains (y, av, ot, V) as activation-Copy ops.

Scales (exactness preserved, folded out on host / in exp):
  wq *= 0.125*32, wk *= 32  -> scores_psum = 1024*s_true, exp scale 2^-10.
  wv *= 32, v_aug ones column = 32.0 -> reciprocal folds the 1/32 back.
Biases handled exactly as baseline (k-bias dropped, q-bias via cexp on
v_aug, v/out-bias folded on host).
"""
import os
import sys
import tempfile

os.environ["NEURON_COMPILE_CACHE_URL"] = tempfile.mkdtemp(prefix="bass_kernel_cache_")

try:
    import concourse  # noqa: F401
except ImportError:
    sys.path.insert(0, "/opt/trn_rl_repo")

import numpy as np
import ml_dtypes

import concourse.bacc as bacc
import concourse.tile as tile
import concourse.mybir as mybir
from concourse.bass_utils import run_bass_kernel_spmd

# ---- problem constants (hardcoded per harness contract) ----
B, T_IN, T_COND, D, H, HD, W = 2, 2048, 1024, 1024, 16, 64, 512
S = T_IN + T_COND            # 3072
SEL = 2 * W                  # 1024 selected keys
NH = 4                       # heads per core
NG = H // NH                 # 4 head groups
NCH = D // 128               # 8 input d-chunks
KT = SEL // 128              # 8 key tiles
QT = S // 128                # 24 query tiles
NSLAB = S // 512             # 6 query slabs
BF16 = mybir.dt.bfloat16
F16 = mybir.dt.float16
F32 = mybir.dt.float32
FP8 = mybir.dt.float8e4
AF = mybir.ActivationFunctionType
ALU = mybir.AluOpType
DR = mybir.MatmulPerfMode.DoubleRow
E4 = ml_dtypes.float8_e4m3

CQ = 32.0                   # extra scale on wq (beyond 0.125)
CK = 32.0                   # scale on wk
CV = 32.0                   # scale on wv; ones column = CV so rec folds it
S_INV = 1.0 / (CQ * CK)     # exp input scale
POOL_HEADS = (0, 1)         # heads whose exp runs DVE-stage + Pool pow
N_WARM = 40                 # PE warmup matmuls

_CACHE = {}


def _build():
    if "nc" in _CACHE:
        return _CACHE["nc"]

    nc = bacc.Bacc("TRN2", target_bir_lowering=False, debug=False,
                   enable_asserts=True, num_devices=8)

    xt8_d = nc.dram_tensor("xt8", (128, NCH, S), FP8, kind="ExternalInput").ap()
    xtr_d = nc.dram_tensor("xtr", (128, NCH, S), FP8, kind="ExternalInput").ap()
    xs8_d = nc.dram_tensor("xs8", (128, NCH, SEL), FP8, kind="ExternalInput").ap()
    xsr_d = nc.dram_tensor("xsr", (128, NCH, SEL), FP8, kind="ExternalInput").ap()
    whiq_d = nc.dram_tensor("whiq", (128, NCH, 256), FP8, kind="ExternalInput").ap()
    wloq_d = nc.dram_tensor("wloq", (128, NCH, 256), FP8, kind="ExternalInput").ap()
    whik_d = nc.dram_tensor("whik", (128, NCH, 256), FP8, kind="ExternalInput").ap()
    wlok_d = nc.dram_tensor("wlok", (128, NCH, 256), FP8, kind="ExternalInput").ap()
    whiv_d = nc.dram_tensor("whiv", (128, NCH, 256), FP8, kind="ExternalInput").ap()
    wlov_d = nc.dram_tensor("wlov", (128, NCH, 256), FP8, kind="ExternalInput").ap()
    wo_d = nc.dram_tensor("wo", (128, 2, D), BF16, kind="ExternalInput").ap()
    cexp_d = nc.dram_tensor("cexp", (128, KT, NH), F32, kind="ExternalInput").ap()
    ident_d = nc.dram_tensor("ident", (128, 128), BF16, kind="ExternalInput").ap()
    y_d = nc.dram_tensor("y", (128, NCH, S), BF16, kind="ExternalOutput").ap()

    with tile.TileContext(nc) as tc:
        with (
            tc.tile_pool(name="const", bufs=1) as cpool,
            tc.tile_pool(name="work", bufs=1) as work,
            tc.tile_pool(name="exps", bufs=6) as epool,
            tc.tile_pool(name="sxp", bufs=4) as sxpool,
            tc.tile_pool(name="osb", bufs=2) as opool,
            tc.tile_pool(name="ysb", bufs=2) as ypool,
            tc.tile_pool(name="ps_s", bufs=4, space="PSUM") as ps_s,   # scores 4x1 banks
            tc.tile_pool(name="ps_qp", bufs=1, space="PSUM") as ps_qp,  # 1 bank
            tc.tile_pool(name="ps_op", bufs=2, space="PSUM") as ps_op,  # 2 banks
            tc.tile_pool(name="ps_av", bufs=1, space="PSUM") as ps_av,  # 1 bank
        ):
            # ---------- input DMAs (SP queue) -- K path first ---------------
            whik = cpool.tile([128, NCH, 256], FP8, tag="whik")
            wlok = cpool.tile([128, NCH, 256], FP8, tag="wlok")
            xs8 = cpool.tile([128, NCH, SEL], FP8, tag="xs8")
            xsr = cpool.tile([128, NCH, SEL], FP8, tag="xsr")
            whiq = cpool.tile([128, NCH, 256], FP8, tag="whiq")
            wloq = cpool.tile([128, NCH, 256], FP8, tag="wloq")
            whiv = cpool.tile([128, NCH, 256], FP8, tag="whiv")
            wlov = cpool.tile([128, NCH, 256], FP8, tag="wlov")
            xt8 = cpool.tile([128, NCH, S], FP8, tag="xt8")
            xtr = cpool.tile([128, NCH, S], FP8, tag="xtr")
            wo = cpool.tile([128, 2, D], BF16, tag="wo")
            cexp = cpool.tile([128, KT, NH], F32, tag="cexp")
            ident = cpool.tile([128, 128], BF16, tag="ident")

            nc.sync.dma_start(whik[:], whik_d[:])
            nc.sync.dma_start(xs8[:, :, 0:512], xs8_d[:, :, 0:512])
            nc.sync.dma_start(xsr[:, :, 0:512], xsr_d[:, :, 0:512])
            nc.sync.dma_start(wlok[:], wlok_d[:])
            nc.sync.dma_start(xs8[:, :, 512:1024], xs8_d[:, :, 512:1024])
            nc.sync.dma_start(xsr[:, :, 512:1024], xsr_d[:, :, 512:1024])
            nc.sync.dma_start(whiq[:], whiq_d[:])
            nc.sync.dma_start(xt8[:, :, 0:512], xt8_d[:, :, 0:512])
            nc.sync.dma_start(wloq[:], wloq_d[:])
            nc.sync.dma_start(xtr[:, :, 0:512], xtr_d[:, :, 0:512])
            nc.sync.dma_start(whiv[:], whiv_d[:])
            nc.sync.dma_start(wlov[:], wlov_d[:])
            nc.sync.dma_start(cexp[:], cexp_d[:])
            nc.sync.dma_start(wo[:], wo_d[:])
            nc.sync.dma_start(ident[:], ident_d[:])
            for sl in range(1, NSLAB):
                nc.sync.dma_start(xt8[:, :, 512 * sl:512 * (sl + 1)],
                                  xt8_d[:, :, 512 * sl:512 * (sl + 1)])
                nc.sync.dma_start(xtr[:, :, 512 * sl:512 * (sl + 1)],
                                  xtr_d[:, :, 512 * sl:512 * (sl + 1)])

            # ---------- persistent tensors ----------
            ktab = [work.tile([128, 2, SEL], FP8, tag=f"kt{i}", name=f"kt{i}")
                    for i in range(2)]                     # slot0=k8, slot1=dup
            qtab = [work.tile([128, 2, S], FP8, tag=f"qt{i}", name=f"qt{i}")
                    for i in range(2)]                     # slot0=q8, slot1=res
            ot = work.tile([128, 2, S], BF16, tag="ot")    # O^T
            v_aug = [work.tile([128, NH, HD + 1], BF16, tag=f"va{kc}",
                               name=f"va{kc}") for kc in range(KT)]
            ebase = work.tile([128, KT // 2, 128], F32, tag="eb")  # e for pow
            zt = work.tile([128, 128], BF16, tag="zt")         # warmup src

            nc.gpsimd.memset(zt[:], 0.0)
            nc.gpsimd.memset(ebase[:], float(np.e))
            for kc in range(KT):
                nc.gpsimd.memset(v_aug[kc][:], CV)

            # ---------- PE warmup: keep PE busy from ~0.5us ----------------
            wm = ps_av.tile([128, NH, HD + 1], F32, tag="av", name="wm")
            for i in range(N_WARM):
                nc.tensor.matmul(wm[:, i % NH, 0:HD], zt[:], zt[:, 0:HD],
                                 start=True, stop=True)

            # ---------- 3-term fp8-DR projection groups --------------------
            def k_proj_group(t, half, pool, tag):
                psk = pool.tile([128, 512], F32, tag=tag, name=f"kp{t}_{half}")
                cols = slice(512 * half, 512 * (half + 1))
                idx = 0
                for (wt, xx) in ((whik, xs8), (whik, xsr), (wlok, xs8)):
                    for c in range(NCH // 2):
                        nc.tensor.matmul(
                            psk[:], wt[:, 2 * c:2 * c + 2, 128 * t:128 * (t + 1)],
                            xx[:, 2 * c:2 * c + 2, cols],
                            start=(idx == 0), stop=(idx == 11), perf_mode=DR)
                        idx += 1
                # drains: k8 (DVE, psum->fp8) + dup (DVE, sbuf->sbuf)
                nc.vector.tensor_copy(ktab[t][:, 0, cols], psk[:])
                nc.vector.tensor_copy(ktab[t][:, 1, cols], ktab[t][:, 0, cols])

            def v_proj_group(kc):
                psv = ps_op.tile([128, 512], F32, tag="op", name=f"vp{kc}")
                idx = 0
                for (xx, wt) in ((xs8, whiv), (xsr, whiv), (xs8, wlov)):
                    for c in range(NCH // 2):
                        nc.tensor.matmul(
                            psv[:, 0:256], xx[:, 2 * c:2 * c + 2, 128 * kc:128 * (kc + 1)],
                            wt[:, 2 * c:2 * c + 2, :],
                            start=(idx == 0), stop=(idx == 11), perf_mode=DR)
                        idx += 1
                nc.vector.tensor_copy(
                    v_aug[kc][:, :, 0:HD],
                    psv[:, 0:256].rearrange("p (h hd) -> p h hd", h=NH))
                for h in range(NH):
                    nc.gpsimd.tensor_scalar(
                        v_aug[kc][:, h, :], v_aug[kc][:, h, :],
                        cexp[:, kc, h:h + 1], None, ALU.mult)

            # Q-proj: 12 DR mms per (t, sl), emitted in chunks.
            qp_state = {}
            QP_TERMS = [(0, 0), (0, 1), (0, 2), (0, 3),    # (term, c): whiq x8
                        (1, 0), (1, 1), (1, 2), (1, 3),    # wloq x8
                        (2, 0), (2, 1), (2, 2), (2, 3)]    # whiq xres

            def q_proj_group(t, sl, lo, hi, pool=None, tag="qp"):
                key = (t, sl)
                if key not in qp_state:
                    qp_state[key] = (pool or ps_qp).tile(
                        [128, 512], F32, tag=tag, name=f"qp{t}_{sl}")
                psq = qp_state[key]
                cols = slice(512 * sl, 512 * (sl + 1))
                for idx in range(lo, hi):
                    term, c = QP_TERMS[idx]
                    wt = (whiq, wloq, whiq)[term]
                    xx = (xt8, xt8, xtr)[term]
                    nc.tensor.matmul(
                        psq[:], wt[:, 2 * c:2 * c + 2, 128 * t:128 * (t + 1)],
                        xx[:, 2 * c:2 * c + 2, cols],
                        start=(idx == 0), stop=(idx == 11), perf_mode=DR)
                if hi == 12:
                    nc.vector.tensor_copy(qtab[t][:, 0, cols], psq[:])
                    nc.vector.scalar_tensor_tensor(
                        qtab[t][:, 1, cols], psq[:], 1.0, qtab[t][:, 0, cols],
                        ALU.mult, ALU.subtract)
                    del qp_state[key]

            # Prologue: K fully (gates first scores), Q slab 0.
            k_proj_group(0, 0, ps_qp, "qp")
            k_proj_group(0, 1, ps_op, "op")
            k_proj_group(1, 0, ps_qp, "qp")
            k_proj_group(1, 1, ps_op, "op")
            q_proj_group(0, 0, 0, 12)
            q_proj_group(1, 0, 0, 12, pool=ps_op, tag="op")

            # ---------- main loop over 128-query tiles ----------------------
            o_prev = None

            def emit_transpose():
                o_sb_p, qtp = o_prev
                for w in range(2):
                    tp = ps_op.tile([128, 128], BF16, tag="op", name=f"tp{qtp}_{w}")
                    nc.tensor.transpose(
                        tp[:], o_sb_p[:, 2 * w:2 * w + 2, :]
                        .rearrange("p a b -> p (a b)"), ident[:])
                    nc.vector.tensor_copy(ot[:, w, 128 * qtp:128 * (qtp + 1)], tp[:])

            def emit_oproj(dt, sl):
                pso = ps_op.tile([128, 512], F32, tag="op", name=f"op{dt}_{sl}")
                for t in range(2):
                    nc.tensor.matmul(
                        pso[:], wo[:, t, 128 * dt:128 * (dt + 1)],
                        ot[:, t, 512 * sl:512 * (sl + 1)],
                        start=(t == 0), stop=(t == 1))
                nc.scalar.copy(y_sb[:, dt, :], pso[:])
                if dt % 2 == 1:
                    nc.sync.dma_start(
                        y_d[:, dt - 1:dt + 1, 512 * sl:512 * (sl + 1)],
                        y_sb[:, dt - 1:dt + 1, :])

            def emit_exp_half(h, hf, st_h, ex_t, qt):
                blk = slice(4 * hf, 4 * hf + 4)
                if h in POOL_HEADS:
                    sx = sxpool.tile([128, KT // 2, 128], F16, tag="sx",
                                     name=f"sx{qt}_{h}_{hf}")
                    nc.vector.tensor_scalar(
                        sx[:], st_h[:], S_INV, None, ALU.mult)
                    nc.gpsimd.tensor_tensor(
                        ex_t[:, blk, :], ebase[:], sx[:], ALU.pow)
                else:
                    nc.scalar.activation(ex_t[:, blk, :], st_h[:], AF.Exp,
                                         scale=S_INV)

            # Pipeline: scores/exp of qt run one qt ahead of AV/normalize so
            # the DVE->Pool exp path has a full qt of latency slack.
            y_sb = None
            ex_prev = None        # ex tiles of qt-1, consumed by AV in qt
            for qt in range(QT + 1):
                sl, r = divmod(qt, 4)
                if r == 2 and 6 <= qt:
                    y_sb = ypool.tile([128, NCH, 512], BF16, tag="ysb",
                                      name=f"ysb{sl}")

                # Q-proj of slab sl+1 in 6-mm chunks (2 chunks per group)
                qp_chunks = []
                if 4 <= qt < 4 * (NSLAB - 1):
                    t = r // 2
                    lo, hi = (0, 6) if r % 2 == 0 else (6, 12)
                    qp_chunks = [(t, sl + 1, lo, hi)]
                u = (qt - 6) // 4
                if 0 <= u:
                    dts = ((4, 5), (6, 7), (0, 1), (2, 3))[r]
                    op_groups = [(dt, u) for dt in dts]
                else:
                    op_groups = []

                ex = [None] * NH
                av = None
                if qt >= 1:
                    av = ps_av.tile([128, NH, HD + 1], F32, tag="av",
                                    name=f"av{qt - 1}")
                for h in range(NH):
                    if qt < QT:
                        X = h // 2
                        o = 64 * (h % 2)
                        ex[h] = epool.tile([128, KT, 128], BF16, tag="ex",
                                           name=f"ex{qt}_{h}")
                        for hf in range(2):
                            st_h = ps_s.tile([128, KT // 2, 128], F32, tag="S",
                                             name=f"s{qt}_{h}_{hf}")
                            for kc in range(KT // 2):
                                kca = 4 * hf + kc
                                nc.tensor.matmul(
                                    st_h[:, kc, :],
                                    ktab[X][o:o + 64, :, 128 * kca:128 * (kca + 1)],
                                    qtab[X][o:o + 64, :, 128 * qt:128 * (qt + 1)],
                                    start=True, stop=True, perf_mode=DR)
                            emit_exp_half(h, hf, st_h, ex[h], qt)

                    if qt == 0:
                        for kc in {1: (0, 1), 2: (2, 3), 3: (4, 5)}.get(h, ()):
                            v_proj_group(kc)
                        continue
                    # AV of qt-1: ACT-path heads (2,3) first, Pool-path later
                    ah = (2, 3, 0, 1)[h]
                    for kc in range(KT):
                        nc.tensor.matmul(
                            av[:, ah, :], ex_prev[ah][:, kc, :],
                            v_aug[kc][:, ah, :],
                            start=(kc == 0), stop=(kc == KT - 1))
                    if qt == 1 and h == 1:
                        q_proj_group(0, 1, 0, 12)
                    if h == 2 and op_groups:
                        emit_oproj(*op_groups[0])
                    if qt == 1 and h == 3:
                        q_proj_group(1, 1, 0, 12)
                    if h == 3 and qp_chunks:
                        q_proj_group(*qp_chunks[0])
                if qt == 0:
                    v_proj_group(6)
                    v_proj_group(7)
                    ex_prev = ex
                    continue
                for g in op_groups[1:]:
                    emit_oproj(*g)
                if o_prev is not None:
                    emit_transpose()

                # normalize qt-1: av -> SBUF (DVE), recip (DVE), scale (Pool).
                av_sb = opool.tile([128, NH, HD + 1], F32, tag="avsb",
                                   name=f"avsb{qt - 1}")
                nc.vector.tensor_copy(av_sb[:], av[:])
                rec = opool.tile([128, NH], F32, tag="rec", name=f"rec{qt - 1}")
                nc.vector.reciprocal(rec[:], av_sb[:, :, HD])
                o_sb = opool.tile([128, NH, HD], BF16, tag="osb",
                                  name=f"o{qt - 1}")
                for h in range(NH):
                    nc.gpsimd.tensor_scalar(
                        o_sb[:, h, :], av_sb[:, h, 0:HD], rec[:, h:h + 1],
                        None, ALU.mult)
                o_prev = (o_sb, qt - 1)
                ex_prev = ex

            # tail: transpose of qt 23, remaining out-proj of slabs 4, 5
            emit_transpose()
            for dt in (6, 7):
                emit_oproj(dt, NSLAB - 2)
            y_sb = ypool.tile([128, NCH, 512], BF16, tag="ysb", name="ysb_tail")
            for dt in range(NCH):
                emit_oproj(dt, NSLAB - 1)

    nc.compile()
    _CACHE["nc"] = nc
    return nc


def _to_pko(a2d, dt=ml_dtypes.bfloat16):
    """(D_in, M) row-major -> [128, D_in//128, M] with d = ko*128 + p."""
    d_in, m = a2d.shape
    return np.ascontiguousarray(
        a2d.reshape(d_in // 128, 128, m).transpose(1, 0, 2).astype(dt))


def _hi_lo(a2d):
    """fp8 hi/lo planes of (D_in, M) array in pko layout."""
    hi = _to_pko(a2d, E4)
    d_in, m = a2d.shape
    hi_f = hi.astype(np.float32)
    lo_f = _to_pko(a2d, np.float32) - hi_f
    return hi, np.ascontiguousarray(lo_f.astype(E4))


def kernel(x, condition, end_inds, in_proj_w, in_proj_b, out_w, out_b):
    nc = _build()

    x = np.asarray(x, dtype=np.float32)
    condition = np.asarray(condition, dtype=np.float32)
    end_inds = np.asarray(end_inds, dtype=np.int32)
    in_proj_w = np.asarray(in_proj_w, dtype=np.float32)
    in_proj_b = np.asarray(in_proj_b, dtype=np.float32)
    out_w = np.asarray(out_w, dtype=np.float32)
    out_b = np.asarray(out_b, dtype=np.float32)

    ident = np.eye(128, dtype=ml_dtypes.bfloat16)
    bo_eff = out_b + out_w @ in_proj_b[2 * D:3 * D]          # v-bias fold

    wq_g, wk_g, wv_g, wo_g, m_g = [], [], [], [], []
    for g in range(NG):
        rows = slice(256 * g, 256 * (g + 1))
        wq_raw = in_proj_w[rows]                              # (256, 1024)
        wk_raw = in_proj_w[D + 256 * g:D + 256 * (g + 1)]
        wv_raw = in_proj_w[2 * D + 256 * g:2 * D + 256 * (g + 1)]
        wq_g.append(_hi_lo(np.ascontiguousarray((0.125 * CQ * wq_raw).T)))
        wk_g.append(_hi_lo(np.ascontiguousarray((CK * wk_raw).T)))
        wv_g.append(_hi_lo(np.ascontiguousarray((CV * wv_raw).T)))
        wo_g.append(_to_pko(np.ascontiguousarray(out_w[:, rows].T)))
        m = np.zeros((D, NH), dtype=np.float32)
        for hl in range(NH):
            bq_h = 0.125 * in_proj_b[256 * g + 64 * hl:256 * g + 64 * hl + 64]
            m[:, hl] = wk_raw[64 * hl:64 * hl + 64].T @ bq_h
        m_g.append(m)

    in_maps = []
    xt_b, xst_b = [], []
    for b in range(B):
        inp = np.concatenate([x[b], condition[b]], axis=0)    # (3072, 1024)
        e = int(end_inds[b])
        sel = np.concatenate([inp[e - W:e], inp[T_IN + e - W:T_IN + e]], axis=0)
        xt_b.append(_hi_lo(np.ascontiguousarray(inp.T)))
        xst_b.append((sel, _hi_lo(np.ascontiguousarray(sel.T))))

    for core in range(8):
        b, g = divmod(core, NG)
        sel, (xs8, xsr) = xst_b[b]
        c = sel @ m_g[g]                                      # (1024, NH)
        cexp = np.exp(c).reshape(KT, 128, NH).transpose(1, 0, 2)
        whiq, wloq = wq_g[g]
        whik, wlok = wk_g[g]
        whiv, wlov = wv_g[g]
        xt8, xtr = xt_b[b]
        in_maps.append({
            "xt8": xt8, "xtr": xtr, "xs8": xs8, "xsr": xsr,
            "whiq": whiq, "wloq": wloq, "whik": whik, "wlok": wlok,
            "whiv": whiv, "wlov": wlov, "wo": wo_g[g],
            "cexp": np.ascontiguousarray(cexp.astype(np.float32)),
            "ident": ident,
        })

    res = run_bass_kernel_spmd(nc, in_maps, core_ids=list(range(8)))

    out = np.zeros((B, S, D), dtype=np.float32)
    for core in range(8):
        b, g = divmod(core, NG)
        yv = np.asarray(res.results[core]["y"]).astype(np.float32)  # [128,8,3072]
        out[b] += yv.transpose(1, 0, 2).reshape(D, S).T       # (3072, 1024)
    out += bo_eff[None, None, :]
    return out
